# revision 15
# baseline (speedup 1.0000x reference)
"""Trainium2 Bass kernel for the channel-attention module.

Reference computation (B=16, N=4096, C=384, H=8, D=48):
    x_in = x @ conv_w.T + conv_b                     # 1x1 conv == linear
    q    = (x_in @ wq.T + bq)  -> [B,H,D,N]
    k, v = (x_in @ wkv.T + bkv) -> 2x [B,H,D,N]
    attn = softmax((q * N**-0.5) @ k^T, axis=-1)      # [B,H,D,D] (over N!)
    out  = attn @ v                                   # [B,H,D,N]
    out  = out.transpose(0,2,1,3).reshape(B,N,C)      # verbatim torch layout
    y    = out @ wp.T + bp

Strategy: pure data parallelism over B across 8 NeuronCores (2 batches per
core), no collectives.  The conv is folded into the q/k/v projections on the
host (w_eff = (w @ conv_w).T, b_eff = b + w @ conv_b), so the device computes
q/k/v straight from x.  All big matmuls run as float32r (full-rate fp32 on
the PE at free-dim >= 256); the tiny per-head S = q^T k matmuls run in bf16.

The awkward transpose(0,2,1,3).reshape is handled exactly with 128-element
flat blocks: flat index (di, h, n) -> block u = 256*di + 32*h + n//128 with
intra-block offset c' = n%128.  Stage 5 (attn @ v) produces AT[c', u] tiles
directly (u = 32*(8*di+h) + t per n-chunk t), and stage 6 reads columns
u = 3*r + j (stride-3 APs) as the K=128 slices of the final projection.
"""

import sys
import types
from contextlib import ExitStack

import numpy as np

import concourse.bass as bass
import concourse.tile as tile
from concourse import bacc, mybir
from concourse.bass_utils import run_bass_kernel_spmd
from concourse.masks import make_identity

B, N, C, H, D = 16, 4096, 384, 8, 48
N_CORES = 8
BPC = B // N_CORES          # batches per core
NW = 512                    # token window for projection matmuls
NWIN = N // NW              # 8 windows
NCHUNK = N // 128           # 32 token chunks of 128
SCALE = float(N) ** -0.5    # 1/64
F32 = mybir.dt.float32
F32R = mybir.dt.float32r
BF16 = mybir.dt.bfloat16


def _install_ntff_hook():
    """The agent image's antenv lacks axon_hooks, so trn_boot's NTFF hook
    registration degrades silently and trace=True would crash.  Recreate the
    module and register the ctypes hook so profiling works."""
    try:
        import antenv

        if "antenv.axon_hooks" in sys.modules:
            return
        mod = types.ModuleType("antenv.axon_hooks")
        mod._hook = None
        mod.set_axon_ntff_profile_hook = lambda h: setattr(mod, "_hook", h)
        mod.get_axon_ntff_profile_hook = lambda: mod._hook
        sys.modules["antenv.axon_hooks"] = mod
        antenv.axon_hooks = mod
        from trn_agent_boot.trn_boot import _ntff_profile_via_ctypes

        mod.set_axon_ntff_profile_hook(
            _ntff_profile_via_ctypes("/opt/axon/libaxon_pjrt.so")
        )
    except Exception:
        pass


def build():
    nc = bacc.Bacc("TRN2", target_bir_lowering=False, debug=False,
                   num_devices=N_CORES)

    # Per-core inputs.  x is pre-transposed on the host to [BPC, C, N].
    xp = nc.declare_dram_parameter("x", [BPC, C, N], BF16, isOutput=False)
    wq_p = nc.declare_dram_parameter("wqT", [C, C], BF16, isOutput=False)
    wk_p = nc.declare_dram_parameter("wkT", [C, C], BF16, isOutput=False)
    wv_p = nc.declare_dram_parameter("wvT", [C, C], BF16, isOutput=False)
    wp_p = nc.declare_dram_parameter("wpT", [C, C], BF16, isOutput=False)
    bq_p = nc.declare_dram_parameter("bq", [C], F32, isOutput=False)
    bk_p = nc.declare_dram_parameter("bk", [C], F32, isOutput=False)
    bv_p = nc.declare_dram_parameter("bv", [C], F32, isOutput=False)
    bp_p = nc.declare_dram_parameter("bp", [C], F32, isOutput=False)
    z_p = nc.declare_dram_parameter("zeros", [128, C], BF16, isOutput=False)
    outp = nc.declare_dram_parameter("out", [BPC, N, C], F32, isOutput=True)

    with tile.TileContext(nc) as tc, ExitStack() as ctx:
        const = ctx.enter_context(tc.tile_pool(name="const", bufs=1))
        xin = ctx.enter_context(tc.tile_pool(name="xin", bufs=3))
        qk = ctx.enter_context(tc.tile_pool(name="qk", bufs=6))
        big = ctx.enter_context(tc.tile_pool(name="big", bufs=2))
        sm = ctx.enter_context(tc.tile_pool(name="sm", bufs=2))
        yout = ctx.enter_context(tc.tile_pool(name="yout", bufs=3))
        ps_big = ctx.enter_context(tc.tile_pool(name="ps_big", bufs=2, space="PSUM"))
        ps_qk = ctx.enter_context(tc.tile_pool(name="ps_qk", bufs=3, space="PSUM"))
        ps_s = ctx.enter_context(tc.tile_pool(name="ps_s", bufs=1, space="PSUM"))
        ps_tr = ctx.enter_context(tc.tile_pool(name="ps_tr", bufs=1, space="PSUM"))

        # ---- constants -----------------------------------------------------
        def load_w(param):
            t = const.tile([128, 3, C], BF16, tag=f"w_{param.name}")
            nc.sync.dma_start(t[:], param.ap().rearrange("(kc p) o -> p kc o", p=128))
            return t

        wq_sb, wk_sb, wv_sb, wp_sb = (load_w(p) for p in (wq_p, wk_p, wv_p, wp_p))

        def load_bias_bcast(param):
            # replicate a [C] bias across all 128 partitions
            t = const.tile([128, C], F32, tag=f"bb_{param.name}")
            ap = param.ap()
            src = bass.AP(tensor=ap.tensor, offset=ap.offset,
                          ap=[[0, 128], *ap.ap])
            nc.sync.dma_start(t[:], src)
            return t

        bq_bc = load_bias_bcast(bq_p)
        bk_bc = load_bias_bcast(bk_p)
        bp_bc = load_bias_bcast(bp_p)

        # bv as per-partition [128, 3] (v is produced channels-on-partitions)
        bv_sb = const.tile([128, 3], F32)
        nc.sync.dma_start(bv_sb[:], bv_p.ap().rearrange("(oc p) -> p oc", p=128))

        id48 = const.tile([48, 48], F32)
        make_identity(nc, id48[:])

        # attn block-diagonal matrices (rhs of stage 5): 3 chunks [128, C].
        # rows c = 48*h + dj, cols q2 = 8*di + h; off-block entries stay 0.
        bd = [const.tile([128, C], BF16, tag=f"bd{i}", name=f"bd{i}")
              for i in range(3)]
        for t in bd:
            nc.sync.dma_start(t[:], z_p.ap()[:, :])

        for b in range(BPC):
            # persistent per-batch accumulators
            vT = big.tile([128, 3, N], BF16, tag="vT")        # [c, n] channels/parts
            at = big.tile([128, C * NCHUNK], BF16, tag="at")  # AT[c', u]
            # u = 256*di + 32*h + t; stage-5 psum columns are (h, di) ordered
            atv = at[:].rearrange("p (d h t) -> p h d t", h=H, t=NCHUNK)
            atr = at[:].rearrange("p (r j) -> p r j", j=3)
            s_ps = ps_s.tile([48, H, 48], F32, tag="s")       # per-head S

            xb = xp.ap()[b].rearrange("(kc p) n -> p kc n", p=128)

            # ---- projections + S accumulation, streamed over N -------------
            for w in range(NWIN):
                xw = xin.tile([128, 3, NW], BF16, tag="xw")
                nc.sync.dma_start(xw[:], xb[:, :, w * NW:(w + 1) * NW])

                # vT window: [c(out) parts, n free]
                for oc in range(3):
                    v_ps = ps_big.tile([128, NW], F32, tag="psbig")
                    for kc in range(3):
                        nc.tensor.matmul(
                            v_ps[:],
                            wv_sb[:, kc, oc * 128:(oc + 1) * 128],
                            xw[:, kc, :],
                            start=(kc == 0), stop=(kc == 2),
                        )
                    nc.scalar.activation(
                        vT[:, oc, w * NW:(w + 1) * NW], v_ps[:],
                        mybir.ActivationFunctionType.Identity,
                        bias=bv_sb[:, oc:oc + 1], scale=1.0,
                    )

                # q/k windows: [n parts, c free], cast to bf16 for S matmuls
                for ns in range(NW // 128):
                    t_chunk = w * (NW // 128) + ns
                    nsl = slice(ns * 128, (ns + 1) * 128)
                    q_sb = qk.tile([128, C], BF16, tag="qsb")
                    k_sb = qk.tile([128, C], BF16, tag="ksb")
                    for dst, wsb, bbc in ((q_sb, wq_sb, bq_bc),
                                          (k_sb, wk_sb, bk_bc)):
                        p_ps = ps_qk.tile([128, C], F32, tag="psqk")
                        for kc in range(3):
                            nc.tensor.matmul(
                                p_ps[:],
                                xw[:, kc, nsl],
                                wsb[:, kc, :],
                                start=(kc == 0), stop=(kc == 2),
                            )
                        nc.vector.tensor_add(dst[:], p_ps[:], bbc[:])
                    # S accumulation, per head (output partitions 0..47)
                    for h in range(H):
                        csl = slice(h * 48, (h + 1) * 48)
                        nc.tensor.matmul(
                            s_ps[:, h, :], q_sb[:, csl], k_sb[:, csl],
                            start=(t_chunk == 0), stop=(t_chunk == NCHUNK - 1),
                        )

            # ---- softmax over dj (no max-subtraction: |S|/64 < 1) ----------
            p_all = sm.tile([48, H, 48], F32, tag="p_all")
            nc.scalar.activation(
                p_all[:], s_ps[:],
                mybir.ActivationFunctionType.Exp,
                bias=0.0, scale=SCALE,
            )
            zsum = sm.tile([48, H], F32, tag="zsum")
            nc.vector.reduce_sum(zsum[:], p_all[:], axis=mybir.AxisListType.X)
            zrec = sm.tile([48, H], F32, tag="zrec")
            nc.vector.reciprocal(zrec[:], zsum[:])
            attn = sm.tile([48, H, 48], F32, tag="attn")
            for h in range(H):
                nc.vector.tensor_scalar_mul(
                    attn[:, h, :], p_all[:, h, :], zrec[:, h:h + 1])

            # ---- transpose each head's attn and scatter into block-diag ----
            tr_ps = ps_tr.tile([48, H, 48], F32, tag="tr")
            for h in range(H):
                nc.tensor.transpose(tr_ps[:, h, :], attn[:, h, :], id48[:])
            attn_t = sm.tile([48, H, 48], BF16, tag="attn_t")
            nc.scalar.activation(
                attn_t[:], tr_ps[:],
                mybir.ActivationFunctionType.Identity, bias=0.0, scale=1.0,
            )
            # scatter attn_t[dj, h, di] -> bd[kc][48h+dj (mod 128), 48h:48h+48]
            # via DMA (engines can't address non-32-aligned partition bases)
            for h in range(H):
                c0 = 48 * h
                dj = 0
                while dj < 48:
                    kc, off = (c0 + dj) // 128, (c0 + dj) % 128
                    cnt = min(48 - dj, 128 - off)
                    nc.sync.dma_start(
                        bd[kc][off:off + cnt, c0:c0 + 48],
                        attn_t[dj:dj + cnt, h, :])
                    dj += cnt

            # ---- stage 5: AT[c', 32*q2 + t] = sum_c vT[c, n] * bd[c, q2] ---
            for t in range(NCHUNK):
                at_ps = ps_qk.tile([128, C], F32, tag="psqk")
                for kc in range(3):
                    nc.tensor.matmul(
                        at_ps[:],
                        vT[:, kc, t * 128:(t + 1) * 128],
                        bd[kc][:],
                        start=(kc == 0), stop=(kc == 2),
                    )
                nc.scalar.activation(
                    atv[:, :, :, t], at_ps[:],
                    mybir.ActivationFunctionType.Identity, bias=0.0, scale=1.0)

            # ---- stage 6: Y[r, o] = sum_j AT[:, 3r+j]^T wpT[128j:, o] + bp -
            for rw in range(NCHUNK):
                y_ps = ps_qk.tile([128, C], F32, tag="psqk")
                for j in range(3):
                    nc.tensor.matmul(
                        y_ps[:],
                        atr[:, rw * 128:(rw + 1) * 128, j],
                        wp_sb[:, j, :],
                        start=(j == 0), stop=(j == 2),
                    )
                y_sb = yout.tile([128, C], F32, tag="ysb")
                nc.vector.tensor_add(y_sb[:], y_ps[:], bp_bc[:])
                nc.sync.dma_start(outp.ap()[b, rw * 128:(rw + 1) * 128, :], y_sb[:])

    nc.compile()
    return nc


_CACHE = {}


def prepare_in_maps(x, conv_w, conv_b, wq, bq, wkv, bkv, wp, bp):
    import ml_dtypes

    bf16 = ml_dtypes.bfloat16
    x = np.ascontiguousarray(x, dtype=np.float32)

    # fold the 1x1 conv into the projections (host-side weight prep)
    wk_w, wv_w = wkv[:C], wkv[C:]
    bk_b, bv_b = bkv[:C], bkv[C:]
    f32 = np.float32
    wqT = np.ascontiguousarray((wq @ conv_w).T, dtype=bf16)
    wkT = np.ascontiguousarray((wk_w @ conv_w).T, dtype=bf16)
    wvT = np.ascontiguousarray((wv_w @ conv_w).T, dtype=bf16)
    wpT = np.ascontiguousarray(wp.T, dtype=bf16)
    bq_e = np.ascontiguousarray(bq + wq @ conv_b, dtype=f32)
    bk_e = np.ascontiguousarray(bk_b + wk_w @ conv_b, dtype=f32)
    bv_e = np.ascontiguousarray(bv_b + wv_w @ conv_b, dtype=f32)
    bp_c = np.ascontiguousarray(bp, dtype=f32)

    xt = np.ascontiguousarray(x.transpose(0, 2, 1).astype(bf16))  # [B, C, N]
    in_maps = []
    for c in range(N_CORES):
        in_maps.append({
            "x": xt[c * BPC:(c + 1) * BPC],
            "wqT": wqT, "wkT": wkT, "wvT": wvT, "wpT": wpT,
            "bq": bq_e, "bk": bk_e, "bv": bv_e, "bp": bp_c,
            "zeros": np.zeros((128, C), dtype=bf16),
        })

    return in_maps


def kernel(x, conv_w, conv_b, wq, bq, wkv, bkv, wp, bp):
    _install_ntff_hook()
    in_maps = prepare_in_maps(x, conv_w, conv_b, wq, bq, wkv, bkv, wp, bp)
    if "nc" not in _CACHE:
        _CACHE["nc"] = build()
    nc = _CACHE["nc"]
    res = run_bass_kernel_spmd(nc, in_maps, core_ids=list(range(N_CORES)))
    out = np.concatenate([res.results[c]["out"] for c in range(N_CORES)], axis=0)
    return out.astype(np.float32)


# revision 17
# speedup vs baseline: 1.0158x; 1.0158x over previous
"""Trainium2 Bass kernel for the channel-attention module.

Reference computation (B=16, N=4096, C=384, H=8, D=48):
    x_in = x @ conv_w.T + conv_b                     # 1x1 conv == linear
    q    = (x_in @ wq.T + bq)  -> [B,H,D,N]
    k, v = (x_in @ wkv.T + bkv) -> 2x [B,H,D,N]
    attn = softmax((q * N**-0.5) @ k^T, axis=-1)      # [B,H,D,D] (over N!)
    out  = attn @ v                                   # [B,H,D,N]
    out  = out.transpose(0,2,1,3).reshape(B,N,C)      # verbatim torch layout
    y    = out @ wp.T + bp

Strategy: pure data parallelism over B across 8 NeuronCores (2 batches per
core), no collectives.  The conv is folded into the q/k/v projections on the
host (w_eff = (w @ conv_w).T, b_eff = b + w @ conv_b), so the device computes
q/k/v straight from x.  All big matmuls run as float32r (full-rate fp32 on
the PE at free-dim >= 256); the tiny per-head S = q^T k matmuls run in bf16.

The awkward transpose(0,2,1,3).reshape is handled exactly with 128-element
flat blocks: flat index (di, h, n) -> block u = 256*di + 32*h + n//128 with
intra-block offset c' = n%128.  Stage 5 (attn @ v) produces AT[c', u] tiles
directly (u = 32*(8*di+h) + t per n-chunk t), and stage 6 reads columns
u = 3*r + j (stride-3 APs) as the K=128 slices of the final projection.
"""

import sys
import types
from contextlib import ExitStack

import numpy as np

import concourse.bass as bass
import concourse.tile as tile
from concourse import bacc, mybir
from concourse.bass_utils import run_bass_kernel_spmd
from concourse.masks import make_identity

B, N, C, H, D = 16, 4096, 384, 8, 48
N_CORES = 8
BPC = B // N_CORES          # batches per core
NW = 512                    # token window for projection matmuls
NWIN = N // NW              # 8 windows
NCHUNK = N // 128           # 32 token chunks of 128
SCALE = float(N) ** -0.5    # 1/64
F32 = mybir.dt.float32
F32R = mybir.dt.float32r
BF16 = mybir.dt.bfloat16


def _install_ntff_hook():
    """The agent image's antenv lacks axon_hooks, so trn_boot's NTFF hook
    registration degrades silently and trace=True would crash.  Recreate the
    module and register the ctypes hook so profiling works."""
    try:
        import antenv

        if "antenv.axon_hooks" in sys.modules:
            return
        mod = types.ModuleType("antenv.axon_hooks")
        mod._hook = None
        mod.set_axon_ntff_profile_hook = lambda h: setattr(mod, "_hook", h)
        mod.get_axon_ntff_profile_hook = lambda: mod._hook
        sys.modules["antenv.axon_hooks"] = mod
        antenv.axon_hooks = mod
        from trn_agent_boot.trn_boot import _ntff_profile_via_ctypes

        mod.set_axon_ntff_profile_hook(
            _ntff_profile_via_ctypes("/opt/axon/libaxon_pjrt.so")
        )
    except Exception:
        pass


def build():
    nc = bacc.Bacc("TRN2", target_bir_lowering=False, debug=False,
                   num_devices=N_CORES)

    # Per-core inputs.  x is pre-transposed on the host to [BPC, C, N].
    xp = nc.declare_dram_parameter("x", [BPC, C, N], BF16, isOutput=False)
    wq_p = nc.declare_dram_parameter("wqT", [C, C], BF16, isOutput=False)
    wk_p = nc.declare_dram_parameter("wkT", [C, C], BF16, isOutput=False)
    wv_p = nc.declare_dram_parameter("wvT", [C, C], BF16, isOutput=False)
    wp_p = nc.declare_dram_parameter("wpT", [C, C], BF16, isOutput=False)
    bq_p = nc.declare_dram_parameter("bq", [C], F32, isOutput=False)
    bk_p = nc.declare_dram_parameter("bk", [C], F32, isOutput=False)
    bv_p = nc.declare_dram_parameter("bv", [C], F32, isOutput=False)
    bp_p = nc.declare_dram_parameter("bp", [C], F32, isOutput=False)
    z_p = nc.declare_dram_parameter("zeros", [128, C], BF16, isOutput=False)
    outp = nc.declare_dram_parameter("out", [BPC, N, C], F32, isOutput=True)

    with tile.TileContext(nc) as tc, ExitStack() as ctx:
        const = ctx.enter_context(tc.tile_pool(name="const", bufs=1))
        xin = ctx.enter_context(tc.tile_pool(name="xin", bufs=3))
        qk = ctx.enter_context(tc.tile_pool(name="qk", bufs=6))
        big = ctx.enter_context(tc.tile_pool(name="big", bufs=2))
        sm = ctx.enter_context(tc.tile_pool(name="sm", bufs=2))
        yout = ctx.enter_context(tc.tile_pool(name="yout", bufs=3))
        ps_big = ctx.enter_context(tc.tile_pool(name="ps_big", bufs=2, space="PSUM"))
        ps_qk = ctx.enter_context(tc.tile_pool(name="ps_qk", bufs=4, space="PSUM"))
        ps_s = ctx.enter_context(tc.tile_pool(name="ps_s", bufs=1, space="PSUM"))
        ps_tr = ctx.enter_context(tc.tile_pool(name="ps_tr", bufs=1, space="PSUM"))

        # ---- constants -----------------------------------------------------
        def load_w(param):
            t = const.tile([128, 3, C], BF16, tag=f"w_{param.name}")
            nc.sync.dma_start(t[:], param.ap().rearrange("(kc p) o -> p kc o", p=128))
            return t

        wq_sb, wk_sb, wv_sb, wp_sb = (load_w(p) for p in (wq_p, wk_p, wv_p, wp_p))

        def load_bias_bcast(param):
            # replicate a [C] bias across all 128 partitions
            t = const.tile([128, C], F32, tag=f"bb_{param.name}")
            ap = param.ap()
            src = bass.AP(tensor=ap.tensor, offset=ap.offset,
                          ap=[[0, 128], *ap.ap])
            nc.sync.dma_start(t[:], src)
            return t

        bq_bc = load_bias_bcast(bq_p)
        bk_bc = load_bias_bcast(bk_p)
        bp_bc = load_bias_bcast(bp_p)

        # bv as per-partition [128, 3] (v is produced channels-on-partitions)
        bv_sb = const.tile([128, 3], F32)
        nc.sync.dma_start(bv_sb[:], bv_p.ap().rearrange("(oc p) -> p oc", p=128))

        id48 = const.tile([48, 48], F32)
        make_identity(nc, id48[:])

        # attn block-diagonal matrices (rhs of stage 5): 3 chunks [128, C].
        # rows c = 48*h + dj, cols q2 = 8*di + h; off-block entries stay 0.
        bd = [const.tile([128, C], BF16, tag=f"bd{i}", name=f"bd{i}")
              for i in range(3)]
        for t in bd:
            nc.sync.dma_start(t[:], z_p.ap()[:, :])

        # PE warm-up: HAM unthrottles after ~3.4us of sustained PE work;
        # burn idle DMA-wait time on dummy matmuls so real matmuls run warm.
        warm_sb = const.tile([128, 1], F32)
        warm_ps = ps_big.tile([128, NW], F32, tag="psbig", name="warm_ps")
        for i in range(24):
            nc.tensor.matmul(warm_ps[:, 0:C], bd[i % 3][:, 0:128], bd[i % 3][:],
                             start=(i == 0), stop=(i == 23))
        nc.vector.tensor_copy(warm_sb[:], warm_ps[:, 0:1])
        nc.sync.dma_start(outp.ap()[0, 0:128, 0:1], warm_sb[:])

        for b in range(BPC):
            # persistent per-batch accumulators
            vT = big.tile([128, 3, N], BF16, tag="vT")        # [c, n] channels/parts
            at = big.tile([128, C * NCHUNK], BF16, tag="at")  # AT[c', u]
            # u = 256*di + 32*h + t; stage-5 psum columns are (h, di) ordered
            atv = at[:].rearrange("p (d h t) -> p h d t", h=H, t=NCHUNK)
            atr = at[:].rearrange("p (r j) -> p r j", j=3)
            s_ps = ps_s.tile([48, H, 48], F32, tag="s")       # per-head S

            xb = xp.ap()[b].rearrange("(kc p) n -> p kc n", p=128)

            # ---- projections + S accumulation, streamed over N -------------
            for w in range(NWIN):
                xw = xin.tile([128, 3, NW], BF16, tag="xw")
                nc.sync.dma_start(xw[:], xb[:, :, w * NW:(w + 1) * NW])

                # vT window: [c(out) parts, n free]
                for oc in range(3):
                    v_ps = ps_big.tile([128, NW], F32, tag="psbig")
                    for kc in range(3):
                        nc.tensor.matmul(
                            v_ps[:],
                            wv_sb[:, kc, oc * 128:(oc + 1) * 128],
                            xw[:, kc, :],
                            start=(kc == 0), stop=(kc == 2),
                        )
                    nc.scalar.activation(
                        vT[:, oc, w * NW:(w + 1) * NW], v_ps[:],
                        mybir.ActivationFunctionType.Identity,
                        bias=bv_sb[:, oc:oc + 1], scale=1.0,
                    )

                # q/k windows: [n parts, c free], cast to bf16 for S matmuls
                for ns in range(NW // 128):
                    t_chunk = w * (NW // 128) + ns
                    nsl = slice(ns * 128, (ns + 1) * 128)
                    q_sb = qk.tile([128, C], BF16, tag="qsb")
                    k_sb = qk.tile([128, C], BF16, tag="ksb")
                    for dst, wsb, bbc in ((q_sb, wq_sb, bq_bc),
                                          (k_sb, wk_sb, bk_bc)):
                        p_ps = ps_qk.tile([128, C], F32, tag="psqk")
                        for kc in range(3):
                            nc.tensor.matmul(
                                p_ps[:],
                                xw[:, kc, nsl],
                                wsb[:, kc, :],
                                start=(kc == 0), stop=(kc == 2),
                            )
                        nc.vector.tensor_add(dst[:], p_ps[:], bbc[:])
                    # S accumulation, per head (output partitions 0..47)
                    for h in range(H):
                        csl = slice(h * 48, (h + 1) * 48)
                        nc.tensor.matmul(
                            s_ps[:, h, :], q_sb[:, csl], k_sb[:, csl],
                            start=(t_chunk == 0), stop=(t_chunk == NCHUNK - 1),
                        )

            # ---- softmax over dj (no max-subtraction: |S|/64 < 1) ----------
            p_all = sm.tile([48, H, 48], F32, tag="p_all")
            nc.scalar.activation(
                p_all[:], s_ps[:],
                mybir.ActivationFunctionType.Exp,
                bias=0.0, scale=SCALE,
            )
            zsum = sm.tile([48, H], F32, tag="zsum")
            nc.vector.reduce_sum(zsum[:], p_all[:], axis=mybir.AxisListType.X)
            zrec = sm.tile([48, H], F32, tag="zrec")
            nc.vector.reciprocal(zrec[:], zsum[:])
            attn = sm.tile([48, H, 48], F32, tag="attn")
            for h in range(H):
                nc.vector.tensor_scalar_mul(
                    attn[:, h, :], p_all[:, h, :], zrec[:, h:h + 1])

            # ---- transpose each head's attn and scatter into block-diag ----
            tr_ps = ps_tr.tile([48, H, 48], F32, tag="tr")
            for h in range(H):
                nc.tensor.transpose(tr_ps[:, h, :], attn[:, h, :], id48[:])
            attn_t = sm.tile([48, H, 48], BF16, tag="attn_t")
            nc.scalar.activation(
                attn_t[:], tr_ps[:],
                mybir.ActivationFunctionType.Identity, bias=0.0, scale=1.0,
            )
            # scatter attn_t[dj, h, di] -> bd[kc][48h+dj (mod 128), 48h:48h+48]
            # via DMA (engines can't address non-32-aligned partition bases)
            for h in range(H):
                c0 = 48 * h
                dj = 0
                while dj < 48:
                    kc, off = (c0 + dj) // 128, (c0 + dj) % 128
                    cnt = min(48 - dj, 128 - off)
                    nc.sync.dma_start(
                        bd[kc][off:off + cnt, c0:c0 + 48],
                        attn_t[dj:dj + cnt, h, :])
                    dj += cnt

            # ---- stage 5: AT[c', 32*q2 + t] = sum_c vT[c, n] * bd[c, q2] ---
            for t in range(NCHUNK):
                at_ps = ps_qk.tile([128, C], F32, tag="psqk")
                for kc in range(3):
                    nc.tensor.matmul(
                        at_ps[:],
                        vT[:, kc, t * 128:(t + 1) * 128],
                        bd[kc][:],
                        start=(kc == 0), stop=(kc == 2),
                    )
                nc.vector.tensor_copy(atv[:, :, :, t], at_ps[:])

            # ---- stage 6: Y[r, o] = sum_j AT[:, 3r+j]^T wpT[128j:, o] + bp -
            for rw in range(NCHUNK):
                y_ps = ps_qk.tile([128, C], F32, tag="psqk")
                for j in range(3):
                    nc.tensor.matmul(
                        y_ps[:],
                        atr[:, rw * 128:(rw + 1) * 128, j],
                        wp_sb[:, j, :],
                        start=(j == 0), stop=(j == 2),
                    )
                y_sb = yout.tile([128, C], F32, tag="ysb")
                nc.vector.tensor_add(y_sb[:], y_ps[:], bp_bc[:])
                nc.sync.dma_start(outp.ap()[b, rw * 128:(rw + 1) * 128, :], y_sb[:])

    nc.compile()
    return nc


_CACHE = {}


def prepare_in_maps(x, conv_w, conv_b, wq, bq, wkv, bkv, wp, bp):
    import ml_dtypes

    bf16 = ml_dtypes.bfloat16
    x = np.ascontiguousarray(x, dtype=np.float32)

    # fold the 1x1 conv into the projections (host-side weight prep)
    wk_w, wv_w = wkv[:C], wkv[C:]
    bk_b, bv_b = bkv[:C], bkv[C:]
    f32 = np.float32
    wqT = np.ascontiguousarray((wq @ conv_w).T, dtype=bf16)
    wkT = np.ascontiguousarray((wk_w @ conv_w).T, dtype=bf16)
    wvT = np.ascontiguousarray((wv_w @ conv_w).T, dtype=bf16)
    wpT = np.ascontiguousarray(wp.T, dtype=bf16)
    bq_e = np.ascontiguousarray(bq + wq @ conv_b, dtype=f32)
    bk_e = np.ascontiguousarray(bk_b + wk_w @ conv_b, dtype=f32)
    bv_e = np.ascontiguousarray(bv_b + wv_w @ conv_b, dtype=f32)
    bp_c = np.ascontiguousarray(bp, dtype=f32)

    xt = np.ascontiguousarray(x.transpose(0, 2, 1).astype(bf16))  # [B, C, N]
    in_maps = []
    for c in range(N_CORES):
        in_maps.append({
            "x": xt[c * BPC:(c + 1) * BPC],
            "wqT": wqT, "wkT": wkT, "wvT": wvT, "wpT": wpT,
            "bq": bq_e, "bk": bk_e, "bv": bv_e, "bp": bp_c,
            "zeros": np.zeros((128, C), dtype=bf16),
        })

    return in_maps


def kernel(x, conv_w, conv_b, wq, bq, wkv, bkv, wp, bp):
    _install_ntff_hook()
    in_maps = prepare_in_maps(x, conv_w, conv_b, wq, bq, wkv, bkv, wp, bp)
    if "nc" not in _CACHE:
        _CACHE["nc"] = build()
    nc = _CACHE["nc"]
    res = run_bass_kernel_spmd(nc, in_maps, core_ids=list(range(N_CORES)))
    out = np.concatenate([res.results[c]["out"] for c in range(N_CORES)], axis=0)
    return out.astype(np.float32)


# revision 20
# speedup vs baseline: 1.2754x; 1.2556x over previous
"""Trainium2 Bass kernel for the channel-attention module.

Reference computation (B=16, N=4096, C=384, H=8, D=48):
    x_in = x @ conv_w.T + conv_b                     # 1x1 conv == linear
    q    = (x_in @ wq.T + bq)  -> [B,H,D,N]
    k, v = (x_in @ wkv.T + bkv) -> 2x [B,H,D,N]
    attn = softmax((q * N**-0.5) @ k^T, axis=-1)      # [B,H,D,D] (over N!)
    out  = attn @ v                                   # [B,H,D,N]
    out  = out.transpose(0,2,1,3).reshape(B,N,C)      # verbatim torch layout
    y    = out @ wp.T + bp

Strategy: pure data parallelism over B across 8 NeuronCores (2 batches per
core), no collectives.  The conv is folded into the q/k/v projections on the
host (w_eff = (w @ conv_w).T, b_eff = b + w @ conv_b), so the device computes
q/k/v straight from x.  All big matmuls run as float32r (full-rate fp32 on
the PE at free-dim >= 256); the tiny per-head S = q^T k matmuls run in bf16.

The awkward transpose(0,2,1,3).reshape is handled exactly with 128-element
flat blocks: flat index (di, h, n) -> block u = 256*di + 32*h + n//128 with
intra-block offset c' = n%128.  Stage 5 (attn @ v) produces AT[c', u] tiles
directly (u = 32*(8*di+h) + t per n-chunk t), and stage 6 reads columns
u = 3*r + j (stride-3 APs) as the K=128 slices of the final projection.
"""

import sys
import types
from contextlib import ExitStack

import numpy as np

import concourse.bass as bass
import concourse.tile as tile
from concourse import bacc, mybir
from concourse.bass_utils import run_bass_kernel_spmd
from concourse.masks import make_identity

B, N, C, H, D = 16, 4096, 384, 8, 48
N_CORES = 8
BPC = B // N_CORES          # batches per core
NW = 512                    # token window for projection matmuls
NWIN = N // NW              # 8 windows
NCHUNK = N // 128           # 32 token chunks of 128
SCALE = float(N) ** -0.5    # 1/64
F32 = mybir.dt.float32
F32R = mybir.dt.float32r
BF16 = mybir.dt.bfloat16


def _install_ntff_hook():
    """The agent image's antenv lacks axon_hooks, so trn_boot's NTFF hook
    registration degrades silently and trace=True would crash.  Recreate the
    module and register the ctypes hook so profiling works."""
    try:
        import antenv

        if "antenv.axon_hooks" in sys.modules:
            return
        mod = types.ModuleType("antenv.axon_hooks")
        mod._hook = None
        mod.set_axon_ntff_profile_hook = lambda h: setattr(mod, "_hook", h)
        mod.get_axon_ntff_profile_hook = lambda: mod._hook
        sys.modules["antenv.axon_hooks"] = mod
        antenv.axon_hooks = mod
        from trn_agent_boot.trn_boot import _ntff_profile_via_ctypes

        mod.set_axon_ntff_profile_hook(
            _ntff_profile_via_ctypes("/opt/axon/libaxon_pjrt.so")
        )
    except Exception:
        pass


def build():
    nc = bacc.Bacc("TRN2", target_bir_lowering=False, debug=False,
                   num_devices=N_CORES)

    # Per-core inputs.  x is pre-transposed on the host to [BPC, C, N].
    xp = nc.declare_dram_parameter("x", [BPC, C, N], BF16, isOutput=False)
    wq_p = nc.declare_dram_parameter("wqT", [C, C], BF16, isOutput=False)
    wk_p = nc.declare_dram_parameter("wkT", [C, C], BF16, isOutput=False)
    wv_p = nc.declare_dram_parameter("wvT", [C, C], BF16, isOutput=False)
    wp_p = nc.declare_dram_parameter("wpT", [C, C], F32R, isOutput=False)
    bq_p = nc.declare_dram_parameter("bq", [C], F32, isOutput=False)
    bk_p = nc.declare_dram_parameter("bk", [C], F32, isOutput=False)
    bv_p = nc.declare_dram_parameter("bv", [C], F32, isOutput=False)
    bp_p = nc.declare_dram_parameter("bp", [C], F32, isOutput=False)
    z_p = nc.declare_dram_parameter("zeros", [128, C], BF16, isOutput=False)
    outp = nc.declare_dram_parameter("out", [BPC, N, C], F32, isOutput=True)

    with tile.TileContext(nc) as tc, ExitStack() as ctx:
        const = ctx.enter_context(tc.tile_pool(name="const", bufs=1))
        xin = ctx.enter_context(tc.tile_pool(name="xin", bufs=3))
        qk = ctx.enter_context(tc.tile_pool(name="qk", bufs=6))
        big = ctx.enter_context(tc.tile_pool(name="big", bufs=2))
        sm = ctx.enter_context(tc.tile_pool(name="sm", bufs=2))
        yout = ctx.enter_context(tc.tile_pool(name="yout", bufs=3))
        ps_big = ctx.enter_context(tc.tile_pool(name="ps_big", bufs=2, space="PSUM"))
        ps_qk = ctx.enter_context(tc.tile_pool(name="ps_qk", bufs=4, space="PSUM"))
        ps_s = ctx.enter_context(tc.tile_pool(name="ps_s", bufs=1, space="PSUM"))
        ps_tr = ctx.enter_context(tc.tile_pool(name="ps_tr", bufs=1, space="PSUM"))

        # ---- constants -----------------------------------------------------
        def load_w(param):
            t = const.tile([128, 3, C], param.dtype, tag=f"w_{param.name}")
            nc.sync.dma_start(t[:], param.ap().rearrange("(kc p) o -> p kc o", p=128))
            return t

        wq_sb, wk_sb, wv_sb, wp_sb = (load_w(p) for p in (wq_p, wk_p, wv_p, wp_p))

        def load_bias_bcast(param):
            # replicate a [C] bias across all 128 partitions
            t = const.tile([128, C], F32, tag=f"bb_{param.name}")
            ap = param.ap()
            src = bass.AP(tensor=ap.tensor, offset=ap.offset,
                          ap=[[0, 128], *ap.ap])
            nc.sync.dma_start(t[:], src)
            return t

        bq_bc = load_bias_bcast(bq_p)
        bk_bc = load_bias_bcast(bk_p)
        bp_bc = load_bias_bcast(bp_p)

        # bv as per-partition [128, 3] (v is produced channels-on-partitions)
        bv_sb = const.tile([128, 3], F32)
        nc.sync.dma_start(bv_sb[:], bv_p.ap().rearrange("(oc p) -> p oc", p=128))

        id48 = const.tile([48, 48], F32)
        make_identity(nc, id48[:])

        # attn block-diagonal matrices (rhs of stage 5): 3 chunks [128, C].
        # rows c = 48*h + dj, cols q2 = 8*di + h; off-block entries stay 0.
        bd = [const.tile([128, C], BF16, tag=f"bd{i}", name=f"bd{i}")
              for i in range(3)]
        for t in bd:
            nc.sync.dma_start(t[:], z_p.ap()[:, :])

        # PE warm-up: HAM unthrottles after ~3.4us of sustained PE work;
        # burn idle DMA-wait time on dummy matmuls so real matmuls run warm.
        warm_sb = const.tile([128, 1], F32)
        warm_ps = ps_big.tile([128, NW], F32, tag="psbig", name="warm_ps")
        for i in range(24):
            nc.tensor.matmul(warm_ps[:, 0:C], bd[i % 3][:, 0:128], bd[i % 3][:],
                             start=(i == 0), stop=(i == 23))
        nc.vector.tensor_copy(warm_sb[:], warm_ps[:, 0:1])
        nc.sync.dma_start(outp.ap()[0, 0:128, 0:1], warm_sb[:])

        for b in range(BPC):
            # persistent per-batch accumulators
            vT = big.tile([128, 3, N], BF16, tag="vT")        # [c, n] channels/parts
            at = big.tile([128, C * NCHUNK], F32R, tag="at")  # AT[c', u]
            # u = 256*di + 32*h + t; stage-5 psum columns are (h, di) ordered
            atv = at[:].rearrange("p (d h t) -> p h d t", h=H, t=NCHUNK)
            atr = at[:].rearrange("p (r j) -> p r j", j=3)
            s_ps = ps_s.tile([48, H, 48], F32, tag="s")       # per-head S

            xb = xp.ap()[b].rearrange("(kc p) n -> p kc n", p=128)

            # ---- projections + S accumulation, streamed over N -------------
            for w in range(NWIN):
                xw = xin.tile([128, 3, NW], BF16, tag="xw")
                nc.sync.dma_start(xw[:], xb[:, :, w * NW:(w + 1) * NW])

                # vT window: [c(out) parts, n free]
                for oc in range(3):
                    v_ps = ps_big.tile([128, NW], F32, tag="psbig")
                    for kc in range(3):
                        nc.tensor.matmul(
                            v_ps[:],
                            wv_sb[:, kc, oc * 128:(oc + 1) * 128],
                            xw[:, kc, :],
                            start=(kc == 0), stop=(kc == 2),
                        )
                    nc.scalar.activation(
                        vT[:, oc, w * NW:(w + 1) * NW], v_ps[:],
                        mybir.ActivationFunctionType.Identity,
                        bias=bv_sb[:, oc:oc + 1], scale=1.0,
                    )

                # q/k windows: [n parts, c free], cast to bf16 for S matmuls
                for ns in range(NW // 128):
                    t_chunk = w * (NW // 128) + ns
                    nsl = slice(ns * 128, (ns + 1) * 128)
                    q_sb = qk.tile([128, C], BF16, tag="qsb")
                    k_sb = qk.tile([128, C], BF16, tag="ksb")
                    for dst, wsb, bbc in ((q_sb, wq_sb, bq_bc),
                                          (k_sb, wk_sb, bk_bc)):
                        p_ps = ps_qk.tile([128, C], F32, tag="psqk")
                        for kc in range(3):
                            nc.tensor.matmul(
                                p_ps[:],
                                xw[:, kc, nsl],
                                wsb[:, kc, :],
                                start=(kc == 0), stop=(kc == 2),
                            )
                        nc.vector.tensor_add(dst[:], p_ps[:], bbc[:])
                    # S accumulation, per head (output partitions 0..47)
                    for h in range(H):
                        csl = slice(h * 48, (h + 1) * 48)
                        nc.tensor.matmul(
                            s_ps[:, h, :], q_sb[:, csl], k_sb[:, csl],
                            start=(t_chunk == 0), stop=(t_chunk == NCHUNK - 1),
                        )

            # ---- softmax over dj (no max-subtraction: |S|/64 < 1) ----------
            p_all = sm.tile([48, H, 48], F32, tag="p_all")
            nc.scalar.activation(
                p_all[:], s_ps[:],
                mybir.ActivationFunctionType.Exp,
                bias=0.0, scale=SCALE,
            )
            zsum = sm.tile([48, H], F32, tag="zsum")
            nc.vector.reduce_sum(zsum[:], p_all[:], axis=mybir.AxisListType.X)
            zrec = sm.tile([48, H], F32, tag="zrec")
            nc.vector.reciprocal(zrec[:], zsum[:])
            attn = sm.tile([48, H, 48], F32, tag="attn")
            for h in range(H):
                nc.vector.tensor_scalar_mul(
                    attn[:, h, :], p_all[:, h, :], zrec[:, h:h + 1])

            # ---- transpose each head's attn and scatter into block-diag ----
            tr_ps = ps_tr.tile([48, H, 48], F32, tag="tr")
            for h in range(H):
                nc.tensor.transpose(tr_ps[:, h, :], attn[:, h, :], id48[:])
            attn_t = sm.tile([48, H, 48], BF16, tag="attn_t")
            nc.scalar.activation(
                attn_t[:], tr_ps[:],
                mybir.ActivationFunctionType.Identity, bias=0.0, scale=1.0,
            )
            # scatter attn_t[dj, h, di] -> bd[kc][48h+dj (mod 128), 48h:48h+48]
            # via DMA (engines can't address non-32-aligned partition bases)
            for h in range(H):
                c0 = 48 * h
                dj = 0
                while dj < 48:
                    kc, off = (c0 + dj) // 128, (c0 + dj) % 128
                    cnt = min(48 - dj, 128 - off)
                    nc.sync.dma_start(
                        bd[kc][off:off + cnt, c0:c0 + 48],
                        attn_t[dj:dj + cnt, h, :])
                    dj += cnt

            # ---- stage 5: AT[c', 32*q2 + t] = sum_c vT[c, n] * bd[c, q2] ---
            for t in range(NCHUNK):
                at_ps = ps_qk.tile([128, C], F32, tag="psqk")
                for kc in range(3):
                    nc.tensor.matmul(
                        at_ps[:],
                        vT[:, kc, t * 128:(t + 1) * 128],
                        bd[kc][:],
                        start=(kc == 0), stop=(kc == 2),
                    )
                nc.vector.tensor_copy(atv[:, 0:4, :, t], at_ps[:, 0:192])
                nc.scalar.activation(
                    atv[:, 4:8, :, t], at_ps[:, 192:384],
                    mybir.ActivationFunctionType.Identity, bias=0.0, scale=1.0)

            # ---- stage 6: Y[r, o] = sum_j AT[:, 3r+j]^T wpT[128j:, o] + bp -
            for rw in range(NCHUNK):
                y_ps = ps_qk.tile([128, C], F32, tag="psqk")
                for j in range(3):
                    nc.tensor.matmul(
                        y_ps[:],
                        atr[:, rw * 128:(rw + 1) * 128, j],
                        wp_sb[:, j, :],
                        start=(j == 0), stop=(j == 2),
                    )
                y_sb = yout.tile([128, C], F32, tag="ysb")
                nc.vector.tensor_add(y_sb[:], y_ps[:], bp_bc[:])
                nc.sync.dma_start(outp.ap()[b, rw * 128:(rw + 1) * 128, :], y_sb[:])

    nc.compile()
    return nc


def _patch_ldw_opt():
    """The env compiles walrus with --enable-ldw-opt=false, which leaves every
    LDWEIGHTS serialized in front of its MATMUL (~77ns tax per matmul).
    Rewrite the flag to true; correctness is validated against the reference
    on every run."""
    import concourse.bass_utils as bu

    if getattr(bu.run_command, "_ldw_patched", False):
        return
    orig = bu.run_command

    def patched(argv, **kw):
        if isinstance(argv, list):
            argv = ["--enable-ldw-opt=true" if a == "--enable-ldw-opt=false"
                    else a for a in argv]
        return orig(argv, **kw)

    patched._ldw_patched = True
    bu.run_command = patched


_CACHE = {}


def prepare_in_maps(x, conv_w, conv_b, wq, bq, wkv, bkv, wp, bp):
    import ml_dtypes

    bf16 = ml_dtypes.bfloat16
    x = np.ascontiguousarray(x, dtype=np.float32)

    # fold the 1x1 conv into the projections (host-side weight prep)
    wk_w, wv_w = wkv[:C], wkv[C:]
    bk_b, bv_b = bkv[:C], bkv[C:]
    f32 = np.float32
    wqT = np.ascontiguousarray((wq @ conv_w).T, dtype=bf16)
    wkT = np.ascontiguousarray((wk_w @ conv_w).T, dtype=bf16)
    wvT = np.ascontiguousarray((wv_w @ conv_w).T, dtype=bf16)
    wpT = np.ascontiguousarray(wp.T, dtype=f32)
    bq_e = np.ascontiguousarray(bq + wq @ conv_b, dtype=f32)
    bk_e = np.ascontiguousarray(bk_b + wk_w @ conv_b, dtype=f32)
    bv_e = np.ascontiguousarray(bv_b + wv_w @ conv_b, dtype=f32)
    bp_c = np.ascontiguousarray(bp, dtype=f32)

    xt = np.ascontiguousarray(x.transpose(0, 2, 1).astype(bf16))  # [B, C, N]
    in_maps = []
    for c in range(N_CORES):
        in_maps.append({
            "x": xt[c * BPC:(c + 1) * BPC],
            "wqT": wqT, "wkT": wkT, "wvT": wvT, "wpT": wpT,
            "bq": bq_e, "bk": bk_e, "bv": bv_e, "bp": bp_c,
            "zeros": np.zeros((128, C), dtype=bf16),
        })

    return in_maps


def kernel(x, conv_w, conv_b, wq, bq, wkv, bkv, wp, bp):
    _install_ntff_hook()
    in_maps = prepare_in_maps(x, conv_w, conv_b, wq, bq, wkv, bkv, wp, bp)
    if "nc" not in _CACHE:
        _CACHE["nc"] = build()
    nc = _CACHE["nc"]
    res = run_bass_kernel_spmd(nc, in_maps, core_ids=list(range(N_CORES)))
    out = np.concatenate([res.results[c]["out"] for c in range(N_CORES)], axis=0)
    return out.astype(np.float32)


# revision 22
# speedup vs baseline: 1.5874x; 1.2447x over previous
"""Trainium2 Bass kernel for the channel-attention module.

Reference computation (B=16, N=4096, C=384, H=8, D=48):
    x_in = x @ conv_w.T + conv_b                      # 1x1 conv == linear
    q    = (x_in @ wq.T + bq)  -> [B,H,D,N]
    k, v = (x_in @ wkv.T + bkv) -> 2x [B,H,D,N]
    attn = softmax((q * N**-0.5) @ k^T, axis=-1)      # [B,H,D,D] (over N!)
    out  = attn @ v                                   # [B,H,D,N]
    out  = out.transpose(0,2,1,3).reshape(B,N,C)      # verbatim torch layout
    y    = out @ wp.T + bp

Strategy: pure data parallelism over B across 8 NeuronCores (2 batches per
core), no collectives.  The conv is folded into the q/k/v projections on the
host (w_eff = (w @ conv_w).T, b_eff = b + w @ conv_b).

q and k are never materialized: since the attention logits contract over N,
S_h = (X wq + 1 bq^T)_h^T (X wk + 1 bk^T)_h
    = wq_h^T G wk_h + u_h (x) bk_h + bq_h (x) (v_h + N bk_h)
with G = X^T X (Gram matrix, accumulated on-chip in PSUM), s = X^T 1 (folded
into G as an extra column via a ones-column appended to x on the host),
u = wq^T s, v = wk^T s.  The rank-1 bias terms are added with K=1 matmuls.

The awkward transpose(0,2,1,3).reshape is handled exactly with 128-element
flat blocks: flat index (di, h, n) -> block u = 256*di + 32*h + n//128 with
intra-block offset c' = n%128.  Stage 5 (attn @ v) produces AT[c', u] tiles
directly through a block-diagonal attn^T matrix (u = 32*(8*di+h) + t per
n-chunk t), and stage 6 reads columns u = 3*r + j (stride-3 APs) as the
K=128 slices of the final projection.  AT stays float32r: 2-byte strided
evacuations are ~2.5x slower on DVE/ACT than 4-byte ones.
"""

import sys
import types
from contextlib import ExitStack

import numpy as np

import concourse.bass as bass
import concourse.tile as tile
from concourse import bacc, mybir
from concourse.bass_utils import run_bass_kernel_spmd
from concourse.masks import make_identity

B, N, C, H, D = 16, 4096, 384, 8, 48
N_CORES = 8
BPC = B // N_CORES          # batches per core
NW = 512                    # token window for the v projection
NWIN = N // NW              # 8 windows
NCHUNK = N // 128           # 32 token chunks of 128
SCALE = float(N) ** -0.5    # 1/64
F32 = mybir.dt.float32
F32R = mybir.dt.float32r
BF16 = mybir.dt.bfloat16


def _install_ntff_hook():
    """The agent image's antenv lacks axon_hooks, so trn_boot's NTFF hook
    registration degrades silently and trace=True would crash.  Recreate the
    module and register the ctypes hook so profiling works."""
    try:
        import antenv

        if "antenv.axon_hooks" in sys.modules:
            return
        mod = types.ModuleType("antenv.axon_hooks")
        mod._hook = None
        mod.set_axon_ntff_profile_hook = lambda h: setattr(mod, "_hook", h)
        mod.get_axon_ntff_profile_hook = lambda: mod._hook
        sys.modules["antenv.axon_hooks"] = mod
        antenv.axon_hooks = mod
        from trn_agent_boot.trn_boot import _ntff_profile_via_ctypes

        mod.set_axon_ntff_profile_hook(
            _ntff_profile_via_ctypes("/opt/axon/libaxon_pjrt.so")
        )
    except Exception:
        pass


def build():
    nc = bacc.Bacc("TRN2", target_bir_lowering=False, debug=False,
                   num_devices=N_CORES)

    # Per-core inputs.  x is pre-transposed on the host to [BPC, C, N]; xn is
    # the natural layout with a ones column appended ([BPC, N, C+1]).
    xp = nc.declare_dram_parameter("x", [BPC, C, N], BF16, isOutput=False)
    xn_p = nc.declare_dram_parameter("xn", [BPC, N, C + 1], BF16, isOutput=False)
    wq_p = nc.declare_dram_parameter("wqT", [C, C], BF16, isOutput=False)
    wk_p = nc.declare_dram_parameter("wkT", [C, C], BF16, isOutput=False)
    wv_p = nc.declare_dram_parameter("wvT", [C, C], BF16, isOutput=False)
    wp_p = nc.declare_dram_parameter("wpT", [C, C], F32R, isOutput=False)
    bq_p = nc.declare_dram_parameter("bq", [1, C], F32R, isOutput=False)
    bk_p = nc.declare_dram_parameter("bk", [1, C], F32R, isOutput=False)
    bv_p = nc.declare_dram_parameter("bv", [C], F32, isOutput=False)
    bp_p = nc.declare_dram_parameter("bp", [C], F32, isOutput=False)
    z_p = nc.declare_dram_parameter("zeros", [128, C], BF16, isOutput=False)
    outp = nc.declare_dram_parameter("out", [BPC, N, C], F32, isOutput=True)

    with tile.TileContext(nc) as tc, ExitStack() as ctx:
        const = ctx.enter_context(tc.tile_pool(name="const", bufs=1))
        xin = ctx.enter_context(tc.tile_pool(name="xin", bufs=3))
        xnp = ctx.enter_context(tc.tile_pool(name="xnp", bufs=3))
        big = ctx.enter_context(tc.tile_pool(name="big", bufs=2))
        big1 = ctx.enter_context(tc.tile_pool(name="big1", bufs=1))
        sm = ctx.enter_context(tc.tile_pool(name="sm", bufs=2))
        yout = ctx.enter_context(tc.tile_pool(name="yout", bufs=3))
        ps_big = ctx.enter_context(tc.tile_pool(name="ps_big", bufs=2, space="PSUM"))
        ps_w = ctx.enter_context(tc.tile_pool(name="ps_w", bufs=5, space="PSUM"))
        ps_s = ctx.enter_context(tc.tile_pool(name="ps_s", bufs=1, space="PSUM"))

        # ---- block-diag tiles + PE warmup first (nothing depends on weights,
        # so this runs during the initial weight/x DMAs and de-throttles HAM).
        bd = [const.tile([128, C], BF16, tag=f"bd{i}", name=f"bd{i}")
              for i in range(3)]
        for t in bd:
            nc.sync.dma_start(t[:], z_p.ap()[:, :])
        warm_sb = const.tile([128, 1], F32)
        warm_ps = ps_big.tile([128, NW], F32, tag="psbig", name="warm_ps")
        for i in range(24):
            nc.tensor.matmul(warm_ps[:, 0:C], bd[i % 3][:, 0:128], bd[i % 3][:],
                             start=(i == 0), stop=(i == 23))
        nc.vector.tensor_copy(warm_sb[:], warm_ps[:, 0:1])
        nc.sync.dma_start(outp.ap()[0, 0:128, 0:1], warm_sb[:])

        # ---- constants -----------------------------------------------------
        def load_w(param):
            t = const.tile([128, 3, C], param.dtype, tag=f"w_{param.name}")
            nc.sync.dma_start(t[:], param.ap().rearrange("(kc p) o -> p kc o", p=128))
            return t

        wq_sb, wk_sb, wv_sb, wp_sb = (load_w(p) for p in (wq_p, wk_p, wv_p, wp_p))

        bq_row = const.tile([1, C], F32R)
        nc.sync.dma_start(bq_row[:], bq_p.ap()[:, :])
        bk_row = const.tile([1, C], F32R)
        nc.sync.dma_start(bk_row[:], bk_p.ap()[:, :])

        # bp replicated across partitions (free-axis bias for the y add)
        bp_bc = const.tile([128, C], F32)
        bp_ap = bp_p.ap()
        nc.sync.dma_start(bp_bc[:], bass.AP(
            tensor=bp_ap.tensor, offset=bp_ap.offset, ap=[[0, 128], *bp_ap.ap]))

        # bv as per-partition [128, 3] (v is produced channels-on-partitions)
        bv_sb = const.tile([128, 3], F32)
        nc.sync.dma_start(bv_sb[:], bv_p.ap().rearrange("(oc p) -> p oc", p=128))

        id48 = const.tile([48, 48], F32)
        make_identity(nc, id48[:])

        for b in range(BPC):
            # persistent per-batch accumulators
            vT = big.tile([128, 3, N], BF16, tag="vT")        # [c, n] channels/parts
            at = big1.tile([128, C * NCHUNK], F32R, tag="at")  # AT[c', u]
            # u = 256*di + 32*h + t; stage-5 psum columns are (h, di) ordered
            atv = at[:].rearrange("p (d h t) -> p h d t", h=H, t=NCHUNK)
            atr = at[:].rearrange("p (r j) -> p r j", j=3)

            xb = xp.ap()[b].rearrange("(kc p) n -> p kc n", p=128)
            xnb = xn_p.ap()[b].rearrange("(t p) c -> p t c", p=128)

            # G = [X | 1]^T-accumulation: G_ps[oc][:, 0:C] = X^T X chunk,
            # G_ps[oc][:, C] = column sums s (thanks to the ones column).
            g_ps = [ps_w.tile([128, C + 1], F32, tag="psw", name=f"g{b}_{i}")
                    for i in range(3)]

            # ---- v projection + Gram accumulation, streamed over N ---------
            for w in range(NWIN):
                xw = xin.tile([128, 3, NW], BF16, tag="xw")
                nc.sync.dma_start(xw[:], xb[:, :, w * NW:(w + 1) * NW])
                xnw = xnp.tile([128, 4, C + 1], BF16, tag="xnw")
                nc.sync.dma_start(xnw[:], xnb[:, 4 * w:4 * w + 4, :])

                for oc in range(3):
                    v_ps = ps_big.tile([128, NW], F32, tag="psbig")
                    for kc in range(3):
                        nc.tensor.matmul(
                            v_ps[:],
                            wv_sb[:, kc, oc * 128:(oc + 1) * 128],
                            xw[:, kc, :],
                            start=(kc == 0), stop=(kc == 2),
                        )
                    nc.scalar.activation(
                        vT[:, oc, w * NW:(w + 1) * NW], v_ps[:],
                        mybir.ActivationFunctionType.Identity,
                        bias=bv_sb[:, oc:oc + 1], scale=1.0,
                    )

                for ns in range(4):
                    t_chunk = 4 * w + ns
                    for oc in range(3):
                        nc.tensor.matmul(
                            g_ps[oc][:],
                            xnw[:, ns, oc * 128:(oc + 1) * 128],
                            xnw[:, ns, :],
                            start=(t_chunk == 0), stop=(t_chunk == NCHUNK - 1),
                        )

            # ---- S_h = wq_h^T G wk_h + rank-1 bias terms -------------------
            g_sb = sm.tile([128, 3, C + 1], BF16, tag="g_sb")
            for oc in range(3):
                nc.scalar.activation(
                    g_sb[:, oc, :], g_ps[oc][:],
                    mybir.ActivationFunctionType.Identity, bias=0.0, scale=1.0)

            # T = G @ wk_eff  (G symmetric, so lhsT slices come straight from G)
            t_sb = sm.tile([128, 3, C], BF16, tag="t_sb")
            for c1 in range(3):
                t_ps = ps_w.tile([128, C], F32, tag="psw", name="t_ps")
                for kc2 in range(3):
                    nc.tensor.matmul(
                        t_ps[:],
                        g_sb[:, kc2, c1 * 128:(c1 + 1) * 128],
                        wk_sb[:, kc2, :],
                        start=(kc2 == 0), stop=(kc2 == 2),
                    )
                nc.scalar.activation(
                    t_sb[:, c1, :], t_ps[:],
                    mybir.ActivationFunctionType.Identity, bias=0.0, scale=1.0)

            # u = wq^T s, v = wk^T s  (s lives in G's ones column)
            uv_sb = []
            for wsb in (wq_sb, wk_sb):
                r_ps = ps_w.tile([1, C], F32, tag="psw", name="r_ps")
                for kc in range(3):
                    nc.tensor.matmul(
                        r_ps[:], g_sb[:, kc, C:C + 1], wsb[:, kc, :],
                        start=(kc == 0), stop=(kc == 2),
                    )
                r_sb = sm.tile([1, C], F32R, tag=f"uv{len(uv_sb)}",
                               name="r_sb")
                nc.vector.tensor_copy(r_sb[:], r_ps[:])
                uv_sb.append(r_sb)
            u_sb, v_sb = uv_sb
            vn_sb = sm.tile([1, C], F32R, tag="vn")
            nc.vector.tensor_scalar_mul(vn_sb[:], bk_row[:], float(N))
            nc.vector.tensor_add(vn_sb[:], vn_sb[:], v_sb[:])

            s_ps = ps_s.tile([48, H, 48], F32, tag="s")
            for h in range(H):
                hsl = slice(48 * h, 48 * (h + 1))
                for kc1 in range(3):
                    nc.tensor.matmul(
                        s_ps[:, h, :], wq_sb[:, kc1, hsl], t_sb[:, kc1, hsl],
                        start=(kc1 == 0), stop=False,
                    )
                nc.tensor.matmul(s_ps[:, h, :], u_sb[:, hsl], bk_row[:, hsl],
                                 start=False, stop=False)
                nc.tensor.matmul(s_ps[:, h, :], bq_row[:, hsl], vn_sb[:, hsl],
                                 start=False, stop=True)

            # ---- softmax over dj (no max-subtraction: |S|/64 < 1) ----------
            p_all = sm.tile([48, H, 48], F32, tag="p_all")
            nc.scalar.activation(
                p_all[:], s_ps[:],
                mybir.ActivationFunctionType.Exp,
                bias=0.0, scale=SCALE,
            )
            zsum = sm.tile([48, H], F32, tag="zsum")
            nc.vector.reduce_sum(zsum[:], p_all[:], axis=mybir.AxisListType.X)
            zrec = sm.tile([48, H], F32, tag="zrec")
            nc.vector.reciprocal(zrec[:], zsum[:])
            attn = sm.tile([48, H, 48], F32, tag="attn")
            for h in range(H):
                nc.vector.tensor_scalar_mul(
                    attn[:, h, :], p_all[:, h, :], zrec[:, h:h + 1])

            # ---- transpose each head's attn and scatter into block-diag ----
            tr_ps = ps_s.tile([48, H, 48], F32, tag="s", name="tr_ps")
            for h in range(H):
                nc.tensor.transpose(tr_ps[:, h, :], attn[:, h, :], id48[:])
            attn_t = sm.tile([48, H, 48], BF16, tag="attn_t")
            nc.scalar.activation(
                attn_t[:], tr_ps[:],
                mybir.ActivationFunctionType.Identity, bias=0.0, scale=1.0,
            )
            # scatter attn_t[dj, h, di] -> bd[kc][48h+dj (mod 128), 48h:48h+48]
            # via DMA (engines can't address non-32-aligned partition bases)
            for h in range(H):
                c0 = 48 * h
                dj = 0
                while dj < 48:
                    kc, off = (c0 + dj) // 128, (c0 + dj) % 128
                    cnt = min(48 - dj, 128 - off)
                    nc.sync.dma_start(
                        bd[kc][off:off + cnt, c0:c0 + 48],
                        attn_t[dj:dj + cnt, h, :])
                    dj += cnt

            # ---- stage 5: AT[c', 32*q2 + t] = sum_c vT[c, n] * bd[c, q2] ---
            for t in range(NCHUNK):
                at_ps = ps_w.tile([128, C], F32, tag="psw", name="at_ps")
                for kc in range(3):
                    nc.tensor.matmul(
                        at_ps[:],
                        vT[:, kc, t * 128:(t + 1) * 128],
                        bd[kc][:],
                        start=(kc == 0), stop=(kc == 2),
                    )
                nc.vector.tensor_copy(atv[:, 0:4, :, t], at_ps[:, 0:192])
                nc.scalar.activation(
                    atv[:, 4:8, :, t], at_ps[:, 192:384],
                    mybir.ActivationFunctionType.Identity, bias=0.0, scale=1.0)

            # ---- stage 6: Y[r, o] = sum_j AT[:, 3r+j]^T wpT[128j:, o] + bp -
            for rw in range(NCHUNK):
                y_ps = ps_w.tile([128, C], F32, tag="psw", name="y_ps")
                for j in range(3):
                    nc.tensor.matmul(
                        y_ps[:],
                        atr[:, rw * 128:(rw + 1) * 128, j],
                        wp_sb[:, j, :],
                        start=(j == 0), stop=(j == 2),
                    )
                y_sb = yout.tile([128, C], F32, tag="ysb")
                nc.vector.tensor_add(y_sb[:], y_ps[:], bp_bc[:])
                nc.sync.dma_start(outp.ap()[b, rw * 128:(rw + 1) * 128, :], y_sb[:])

    nc.compile()
    return nc


_CACHE = {}


def prepare_in_maps(x, conv_w, conv_b, wq, bq, wkv, bkv, wp, bp):
    import ml_dtypes

    bf16 = ml_dtypes.bfloat16
    f32 = np.float32
    x = np.ascontiguousarray(x, dtype=f32)

    # fold the 1x1 conv into the projections (host-side weight prep)
    wk_w, wv_w = wkv[:C], wkv[C:]
    bk_b, bv_b = bkv[:C], bkv[C:]
    wqT = np.ascontiguousarray((wq @ conv_w).T, dtype=bf16)
    wkT = np.ascontiguousarray((wk_w @ conv_w).T, dtype=bf16)
    wvT = np.ascontiguousarray((wv_w @ conv_w).T, dtype=bf16)
    wpT = np.ascontiguousarray(wp.T, dtype=f32)
    bq_e = np.ascontiguousarray((bq + wq @ conv_b).reshape(1, C), dtype=f32)
    bk_e = np.ascontiguousarray((bk_b + wk_w @ conv_b).reshape(1, C), dtype=f32)
    bv_e = np.ascontiguousarray(bv_b + wv_w @ conv_b, dtype=f32)
    bp_c = np.ascontiguousarray(bp, dtype=f32)

    xt = np.ascontiguousarray(x.transpose(0, 2, 1).astype(bf16))  # [B, C, N]
    xb = x.astype(bf16)
    xn = np.ascontiguousarray(
        np.concatenate([xb, np.ones((B, N, 1), dtype=bf16)], axis=2))

    in_maps = []
    for c in range(N_CORES):
        in_maps.append({
            "x": xt[c * BPC:(c + 1) * BPC],
            "xn": xn[c * BPC:(c + 1) * BPC],
            "wqT": wqT, "wkT": wkT, "wvT": wvT, "wpT": wpT,
            "bq": bq_e, "bk": bk_e, "bv": bv_e, "bp": bp_c,
            "zeros": np.zeros((128, C), dtype=bf16),
        })

    return in_maps


def kernel(x, conv_w, conv_b, wq, bq, wkv, bkv, wp, bp):
    _install_ntff_hook()
    in_maps = prepare_in_maps(x, conv_w, conv_b, wq, bq, wkv, bkv, wp, bp)
    if "nc" not in _CACHE:
        _CACHE["nc"] = build()
    nc = _CACHE["nc"]
    res = run_bass_kernel_spmd(nc, in_maps, core_ids=list(range(N_CORES)))
    out = np.concatenate([res.results[c]["out"] for c in range(N_CORES)], axis=0)
    return out.astype(np.float32)
